# revision 3
# baseline (speedup 1.0000x reference)
"""Trainium2 Bass kernel for a pre-norm transformer decoder layer.

Full inputs in, full output out. Internally: 8-way data-parallel over
tokens (batch 2 x 4 query-slices of 512 tokens). Each core redundantly
computes K/V for its batch's full 2048-token sequence (no collectives),
and owns 512 query tokens end-to-end (attention, out-proj, MLP).

Shapes: x (2, 2048, 1024), 16 heads, dk=64, d_ff=2048, eps=1e-5.
"""
import threading

import numpy as np
import ml_dtypes

import concourse.mybir as mybir
import concourse.tile as tile
from concourse import bacc
from concourse.bass_utils import run_bass_kernel_spmd
from concourse.masks import make_identity
from contextlib import ExitStack

F32 = mybir.dt.float32
BF16 = mybir.dt.bfloat16
AF = mybir.ActivationFunctionType
OP = mybir.AluOpType

B, S, D = 2, 2048, 1024
H, DK, FF = 16, 64, 2048
EPS = 1e-5
NCORES = 8
SQ = S * B // NCORES          # 512 own query tokens per core
ND = D // 128                 # 8 feature tiles
NT = S // 128                 # 16 sequence tiles
NTQ = SQ // 128               # 4 own-token tiles
NF = FF // 128                # 16 ff tiles
NKC = S // 512                # 4 key chunks of 512

_BF = ml_dtypes.bfloat16


def _build_nc():
    nc = bacc.Bacc("TRN2", target_bir_lowering=False, debug=False,
                   num_devices=NCORES)

    x = nc.dram_tensor("x", [S, D], F32, kind="ExternalInput").ap()
    wqt = nc.dram_tensor("wqt", [D, D], BF16, kind="ExternalInput").ap()
    wkt = nc.dram_tensor("wkt", [D, D], BF16, kind="ExternalInput").ap()
    wvt = nc.dram_tensor("wvt", [D, D], BF16, kind="ExternalInput").ap()
    wot = nc.dram_tensor("wot", [D, D], BF16, kind="ExternalInput").ap()
    w1t = nc.dram_tensor("w1t", [D, FF], BF16, kind="ExternalInput").ap()
    w2t = nc.dram_tensor("w2t", [FF, D], BF16, kind="ExternalInput").ap()
    bqd = nc.dram_tensor("bq", [128, ND], F32, kind="ExternalInput").ap()
    bkd = nc.dram_tensor("bk", [128, ND], F32, kind="ExternalInput").ap()
    bvd = nc.dram_tensor("bv", [1, D], F32, kind="ExternalInput").ap()
    bod = nc.dram_tensor("bo", [128, ND], F32, kind="ExternalInput").ap()
    b1d = nc.dram_tensor("b1", [128, NF], F32, kind="ExternalInput").ap()
    b2d = nc.dram_tensor("b2", [128, ND], F32, kind="ExternalInput").ap()
    outd = nc.dram_tensor("out", [SQ, D], F32, kind="ExternalOutput").ap()

    with tile.TileContext(nc) as tc, ExitStack() as ctx:
        # ---- whole-kernel pools (stack level 0) ----
        const = ctx.enter_context(tc.tile_pool(name="const", bufs=1))
        pa = ctx.enter_context(tc.tile_pool(name="pa", bufs=4, space="PSUM"))
        pt = ctx.enter_context(tc.tile_pool(name="pt", bufs=4, space="PSUM"))
        wA = ctx.enter_context(tc.tile_pool(name="wA", bufs=16))
        xrp = ctx.enter_context(tc.tile_pool(name="xrp", bufs=1))

        ident_bf = const.tile([128, 128], BF16, tag="idbf")
        make_identity(nc, ident_bf)
        ident_f = const.tile([128, 128], F32, tag="idf")
        make_identity(nc, ident_f)
        eps_sb = const.tile([128, 1], F32, tag="eps")
        nc.vector.memset(eps_sb, EPS)
        bq_sb = const.tile([128, ND], F32, tag="bq")
        nc.sync.dma_start(out=bq_sb, in_=bqd)
        bk_sb = const.tile([128, ND], F32, tag="bk")
        nc.sync.dma_start(out=bk_sb, in_=bkd)
        bo_sb = const.tile([128, ND], F32, tag="bo")
        nc.sync.dma_start(out=bo_sb, in_=bod)
        b1_sb = const.tile([128, NF], F32, tag="b1")
        nc.sync.dma_start(out=b1_sb, in_=b1d)
        b2_sb = const.tile([128, ND], F32, tag="b2")
        nc.sync.dma_start(out=b2_sb, in_=b2d)
        bv_row = const.tile([1, D], F32, tag="bvr")
        nc.sync.dma_start(out=bv_row, in_=bvd)
        bv_bc = const.tile([128, D], F32, tag="bvb")
        nc.gpsimd.partition_broadcast(bv_bc, bv_row)

        x_res = [xrp.tile([128, D], F32, tag=f"xr{t}", name=f"xr{t}")
                 for t in range(NTQ)]

        # ---- level 1: K/V/Q/O live from QKV through out-projection ----
        ctxKVQ = ExitStack()
        kp = ctxKVQ.enter_context(tc.tile_pool(name="kp", bufs=1))
        vp = ctxKVQ.enter_context(tc.tile_pool(name="vp", bufs=1))
        qp = ctxKVQ.enter_context(tc.tile_pool(name="qp", bufs=1))
        op_ = ctxKVQ.enter_context(tc.tile_pool(name="op", bufs=1))
        k_fm = [kp.tile([128, S], BF16, tag=f"k{d}", name=f"k{d}")
                for d in range(ND)]
        v_aug = [vp.tile([128, H, DK + 1], BF16, tag=f"v{t}", name=f"v{t}")
                 for t in range(NT)]
        q_fm = [qp.tile([128, SQ], BF16, tag=f"q{d}", name=f"q{d}")
                for d in range(ND)]
        o_fm = [op_.tile([128, SQ], BF16, tag=f"o{j}", name=f"o{j}")
                for j in range(ND)]

        # ---- level 2: LN1 + z (freed right after QKV) ----
        ctxZ = ExitStack()
        zp = ctxZ.enter_context(tc.tile_pool(name="zp", bufs=1))
        xin = ctxZ.enter_context(tc.tile_pool(name="xin", bufs=2))
        lns = ctxZ.enter_context(tc.tile_pool(name="lns", bufs=3))
        z_fm = [zp.tile([128, S], BF16, tag=f"z{d}", name=f"z{d}")
                for d in range(ND)]

        for t in range(NT):
            x_t = xin.tile([128, D], F32, tag="x")
            nc.sync.dma_start(out=x_t, in_=x[t * 128:(t + 1) * 128, :])
            st = lns.tile([128, 2, 6], F32, tag="st")
            nc.vector.bn_stats(st[:, 0, :], x_t[:, 0:512])
            nc.vector.bn_stats(st[:, 1, :], x_t[:, 512:1024])
            mv = lns.tile([128, 2], F32, tag="mv")
            nc.vector.bn_aggr(mv, st)
            sq = lns.tile([128, 1], F32, tag="sq")
            nc.scalar.activation(sq, mv[:, 1:2], AF.Sqrt, bias=eps_sb[:, 0:1],
                                 scale=1.0)
            rstd = lns.tile([128, 1], F32, tag="rstd")
            nc.vector.reciprocal(rstd, sq)
            z_tm = lns.tile([128, D], BF16, tag="ztm")
            nc.vector.tensor_scalar(z_tm, x_t, mv[:, 0:1], rstd,
                                    op0=OP.subtract, op1=OP.mult)
            for d in range(ND):
                tp = pt.tile([128, 128], BF16, tag="pt")
                nc.tensor.transpose(tp, z_tm[:, d * 128:(d + 1) * 128],
                                    ident_bf)
                dst = z_fm[d][:, t * 128:(t + 1) * 128]
                if d % 2 == 0:
                    nc.scalar.copy(dst, tp)
                else:
                    nc.vector.tensor_copy(dst, tp)

        # ---- QKV projections ----
        wq_sb = []
        for d in range(ND):
            w = wA.tile([128, D], BF16, tag="wA", name=f"wq{d}")
            nc.sync.dma_start(out=w, in_=wqt[d * 128:(d + 1) * 128, :])
            wq_sb.append(w)
        wk_sb = []
        for d in range(ND):
            w = wA.tile([128, D], BF16, tag="wA", name=f"wk{d}")
            nc.sync.dma_start(out=w, in_=wkt[d * 128:(d + 1) * 128, :])
            wk_sb.append(w)

        # Q: own 512 tokens
        for j in range(ND):
            pq = pa.tile([128, 512], F32, tag="pa")
            for d in range(ND):
                nc.tensor.matmul(pq, wq_sb[d][:, j * 128:(j + 1) * 128],
                                 z_fm[d][:, 0:SQ],
                                 start=(d == 0), stop=(d == ND - 1))
            nc.vector.tensor_scalar(q_fm[j], pq, bq_sb[:, j:j + 1], None,
                                    op0=OP.add)

        # K: full 2048 tokens
        for j in range(ND):
            for c in range(NKC):
                pk = pa.tile([128, 512], F32, tag="pa")
                for d in range(ND):
                    nc.tensor.matmul(pk, wk_sb[d][:, j * 128:(j + 1) * 128],
                                     z_fm[d][:, c * 512:(c + 1) * 512],
                                     start=(d == 0), stop=(d == ND - 1))
                nc.vector.tensor_scalar(k_fm[j][:, c * 512:(c + 1) * 512], pk,
                                        bk_sb[:, j:j + 1], None, op0=OP.add)

        wv_sb = []
        for d in range(ND):
            w = wA.tile([128, D], BF16, tag="wA", name=f"wv{d}")
            nc.sync.dma_start(out=w, in_=wvt[d * 128:(d + 1) * 128, :])
            wv_sb.append(w)

        # V: token-major with appended ones column (softmax denominator)
        for t in range(NT):
            nc.vector.memset(v_aug[t][:, :, DK:DK + 1], 1.0)
            for c in range(2):
                pv = pa.tile([128, 512], F32, tag="pa")
                for d in range(ND):
                    nc.tensor.matmul(pv, z_fm[d][:, t * 128:(t + 1) * 128],
                                     wv_sb[d][:, c * 512:(c + 1) * 512],
                                     start=(d == 0), stop=(d == ND - 1))
                nc.vector.tensor_add(
                    v_aug[t][:, c * 8:(c + 1) * 8, 0:DK],
                    pv.rearrange("p (h d) -> p h d", h=8),
                    bv_bc[:, c * 512:(c + 1) * 512].rearrange(
                        "p (h d) -> p h d", h=8))
        ctxZ.close()

        # ---- attention (transposed scores, no max subtraction) ----
        ctxATT = ExitStack()
        stp = ctxATT.enter_context(tc.tile_pool(name="stp", bufs=4))
        rp = ctxATT.enter_context(tc.tile_pool(name="rp", bufs=2))

        wo_sb = []
        for d in range(ND):
            w = wA.tile([128, D], BF16, tag="wA", name=f"wo{d}")
            nc.sync.dma_start(out=w, in_=wot[d * 128:(d + 1) * 128, :])
            wo_sb.append(w)

        for hp in range(H // 2):
            ppv = [pa.tile([DK + 1, 512], F32, tag="pa", name=f"ppv{hp}_{i}")
                   for i in range(2)]
            sts = {}
            for kt in range(NT + 1):
                if kt < NT:
                    for s in range(2):
                        base = s * 64
                        psc = pa.tile([128, 512], F32, tag="pa")
                        nc.tensor.matmul(
                            psc,
                            k_fm[hp][base:base + 64, kt * 128:(kt + 1) * 128],
                            q_fm[hp][base:base + 64, :],
                            start=True, stop=True)
                        st_t = stp.tile([128, 512], BF16, tag="st")
                        nc.scalar.activation(st_t, psc, AF.Exp, bias=0.0,
                                             scale=0.125)
                        sts[(kt, s)] = st_t
                if kt > 0:
                    for s in range(2):
                        h = 2 * hp + s
                        nc.tensor.matmul(ppv[s], v_aug[kt - 1][:, h, :],
                                         sts.pop((kt - 1, s)),
                                         start=(kt == 1), stop=(kt == NT))
            for s in range(2):
                rh = rp.tile([1, 512], F32, tag="rh")
                nc.vector.reciprocal(rh, ppv[s][DK:DK + 1, :])
                rb = rp.tile([64, 512], F32, tag="rb")
                nc.gpsimd.partition_broadcast(rb, rh)
                nc.vector.tensor_mul(o_fm[hp][s * 64:(s + 1) * 64, :],
                                     ppv[s][0:DK, :], rb)
        ctxATT.close()

        # ---- output projection + residual (x_res = xq + y) ----
        ctxY = ExitStack()
        xqy = ctxY.enter_context(tc.tile_pool(name="xqy", bufs=1))
        ytp = ctxY.enter_context(tc.tile_pool(name="ytp", bufs=2))
        xq = [xqy.tile([128, D], F32, tag=f"xq{t}", name=f"xq{t}")
              for t in range(NTQ)]
        for t in range(NTQ):
            nc.sync.dma_start(out=xq[t], in_=x[t * 128:(t + 1) * 128, :])

        for o in range(ND):
            py = pa.tile([128, 512], F32, tag="pa")
            for j in range(ND):
                nc.tensor.matmul(py, wo_sb[j][:, o * 128:(o + 1) * 128],
                                 o_fm[j], start=(j == 0), stop=(j == ND - 1))
            y_tmp = ytp.tile([128, 512], F32, tag="yt")
            nc.vector.tensor_scalar(y_tmp, py, bo_sb[:, o:o + 1], None,
                                    op0=OP.add)
            for t in range(NTQ):
                tp = pt.tile([128, 128], F32, tag="pt")
                nc.tensor.transpose(tp, y_tmp[:, t * 128:(t + 1) * 128],
                                    ident_f)
                nc.vector.tensor_add(x_res[t][:, o * 128:(o + 1) * 128], tp,
                                     xq[t][:, o * 128:(o + 1) * 128])
        ctxY.close()
        ctxKVQ.close()

        # ---- LN2 + MLP + residual ----
        ctxMLP = ExitStack()
        z2p = ctxMLP.enter_context(tc.tile_pool(name="z2p", bufs=1))
        hp_ = ctxMLP.enter_context(tc.tile_pool(name="hp", bufs=1))
        wB = ctxMLP.enter_context(tc.tile_pool(name="wB", bufs=9))
        lns2 = ctxMLP.enter_context(tc.tile_pool(name="lns2", bufs=3))
        y2tp = ctxMLP.enter_context(tc.tile_pool(name="y2tp", bufs=2))
        outp = ctxMLP.enter_context(tc.tile_pool(name="outp", bufs=1))
        z2_fm = [z2p.tile([128, SQ], BF16, tag=f"z2{d}", name=f"z2{d}")
                 for d in range(ND)]
        h_fm = [hp_.tile([128, SQ], BF16, tag=f"h{f}", name=f"h{f}")
                for f in range(NF)]
        out_tm = [outp.tile([128, D], F32, tag=f"ot{t}", name=f"ot{t}")
                  for t in range(NTQ)]

        w1_sb = []
        for d in range(ND):
            w = wB.tile([128, FF], BF16, tag="wB", name=f"w1_{d}")
            nc.sync.dma_start(out=w, in_=w1t[d * 128:(d + 1) * 128, :])
            w1_sb.append(w)

        for t in range(NTQ):
            st = lns2.tile([128, 2, 6], F32, tag="st2")
            nc.vector.bn_stats(st[:, 0, :], x_res[t][:, 0:512])
            nc.vector.bn_stats(st[:, 1, :], x_res[t][:, 512:1024])
            mv = lns2.tile([128, 2], F32, tag="mv2")
            nc.vector.bn_aggr(mv, st)
            sq = lns2.tile([128, 1], F32, tag="sq2")
            nc.scalar.activation(sq, mv[:, 1:2], AF.Sqrt, bias=eps_sb[:, 0:1],
                                 scale=1.0)
            rstd = lns2.tile([128, 1], F32, tag="rstd2")
            nc.vector.reciprocal(rstd, sq)
            z2_tm = lns2.tile([128, D], BF16, tag="z2tm")
            nc.vector.tensor_scalar(z2_tm, x_res[t], mv[:, 0:1], rstd,
                                    op0=OP.subtract, op1=OP.mult)
            for d in range(ND):
                tp = pt.tile([128, 128], BF16, tag="pt")
                nc.tensor.transpose(tp, z2_tm[:, d * 128:(d + 1) * 128],
                                    ident_bf)
                dst = z2_fm[d][:, t * 128:(t + 1) * 128]
                if d % 2 == 0:
                    nc.scalar.copy(dst, tp)
                else:
                    nc.vector.tensor_copy(dst, tp)

        w2_sb = []
        for f in range(NF):
            w = wA.tile([128, D], BF16, tag="wA", name=f"w2_{f}")
            nc.sync.dma_start(out=w, in_=w2t[f * 128:(f + 1) * 128, :])
            w2_sb.append(w)

        for f in range(NF):
            ph = pa.tile([128, 512], F32, tag="pa")
            for d in range(ND):
                nc.tensor.matmul(ph, w1_sb[d][:, f * 128:(f + 1) * 128],
                                 z2_fm[d], start=(d == 0), stop=(d == ND - 1))
            nc.scalar.activation(h_fm[f], ph, AF.Relu, bias=b1_sb[:, f:f + 1],
                                 scale=1.0)

        for o in range(ND):
            p2 = pa.tile([128, 512], F32, tag="pa")
            for f in range(NF):
                nc.tensor.matmul(p2, w2_sb[f][:, o * 128:(o + 1) * 128],
                                 h_fm[f], start=(f == 0), stop=(f == NF - 1))
            y2_tmp = y2tp.tile([128, 512], F32, tag="y2t")
            nc.vector.tensor_scalar(y2_tmp, p2, b2_sb[:, o:o + 1], None,
                                    op0=OP.add)
            for t in range(NTQ):
                tp = pt.tile([128, 128], F32, tag="pt")
                nc.tensor.transpose(tp, y2_tmp[:, t * 128:(t + 1) * 128],
                                    ident_f)
                nc.vector.tensor_add(out_tm[t][:, o * 128:(o + 1) * 128], tp,
                                     x_res[t][:, o * 128:(o + 1) * 128])
        for t in range(NTQ):
            nc.sync.dma_start(out=outd[t * 128:(t + 1) * 128, :],
                              in_=out_tm[t])
        ctxMLP.close()

    nc.compile()
    return nc


_LOCK = threading.Lock()
_NC = None


def _get_nc():
    global _NC
    with _LOCK:
        if _NC is None:
            _NC = _build_nc()
    return _NC


def _prep_inputs(inputs):
    x = np.asarray(inputs["x"], np.float32)
    g1 = np.asarray(inputs["ln1_g"], np.float32)
    b1v = np.asarray(inputs["ln1_b"], np.float32)
    g2 = np.asarray(inputs["ln2_g"], np.float32)
    b2v = np.asarray(inputs["ln2_b"], np.float32)
    wq = np.asarray(inputs["wq"], np.float32)
    wk = np.asarray(inputs["wk"], np.float32)
    wv = np.asarray(inputs["wv"], np.float32)
    wo = np.asarray(inputs["wo"], np.float32)
    w1 = np.asarray(inputs["w1"], np.float32)
    w2 = np.asarray(inputs["w2"], np.float32)

    shared = {
        "wqt": np.ascontiguousarray((g1[:, None] * wq.T)).astype(_BF),
        "wkt": np.ascontiguousarray((g1[:, None] * wk.T)).astype(_BF),
        "wvt": np.ascontiguousarray((g1[:, None] * wv.T)).astype(_BF),
        "wot": np.ascontiguousarray(wo.T).astype(_BF),
        "w1t": np.ascontiguousarray((g2[:, None] * w1.T)).astype(_BF),
        "w2t": np.ascontiguousarray(w2.T).astype(_BF),
        "bq": np.ascontiguousarray(
            (inputs["bq"] + wq @ b1v).astype(np.float32).reshape(ND, 128).T),
        "bk": np.ascontiguousarray(
            (inputs["bk"] + wk @ b1v).astype(np.float32).reshape(ND, 128).T),
        "bv": (inputs["bv"] + wv @ b1v).astype(np.float32).reshape(1, D),
        "bo": np.ascontiguousarray(
            np.asarray(inputs["bo"], np.float32).reshape(ND, 128).T),
        "b1": np.ascontiguousarray(
            (inputs["b1"] + w1 @ b2v).astype(np.float32).reshape(NF, 128).T),
        "b2": np.ascontiguousarray(
            np.asarray(inputs["b2"], np.float32).reshape(ND, 128).T),
    }

    in_maps = []
    for c in range(NCORES):
        b = c // (NCORES // B)
        qoff = (c % (NCORES // B)) * SQ
        xb = x[b]
        x_perm = np.ascontiguousarray(
            np.concatenate([xb[qoff:qoff + SQ], xb[:qoff], xb[qoff + SQ:]],
                           axis=0))
        m = dict(shared)
        m["x"] = x_perm
        in_maps.append(m)
    return in_maps


def _run(inputs, trace=False):
    nc = _get_nc()
    in_maps = _prep_inputs(inputs)
    res = run_bass_kernel_spmd(nc, in_maps, core_ids=list(range(NCORES)),
                               trace=trace)
    out = np.empty((B, S, D), np.float32)
    for c in range(NCORES):
        b = c // (NCORES // B)
        qoff = (c % (NCORES // B)) * SQ
        out[b, qoff:qoff + SQ] = res.results[c]["out"]
    return out, res


def kernel(**inputs):
    out, _ = _run(inputs, trace=False)
    return out


# revision 7
# speedup vs baseline: 1.0754x; 1.0754x over previous
"""Trainium2 Bass kernel for a pre-norm transformer decoder layer.

Full inputs in, full output out. Internally: 8-way data-parallel over
tokens (batch 2 x 4 query-slices of 512 tokens). Each core redundantly
computes K/V for its batch's full 2048-token sequence (no collectives),
and owns 512 query tokens end-to-end (attention, out-proj, MLP).

Shapes: x (2, 2048, 1024), 16 heads, dk=64, d_ff=2048, eps=1e-5.
"""
import threading

import numpy as np
import ml_dtypes

import concourse.mybir as mybir
import concourse.tile as tile
from concourse import bacc
from concourse.bass_utils import run_bass_kernel_spmd
from concourse.masks import make_identity
from contextlib import ExitStack

F32 = mybir.dt.float32
BF16 = mybir.dt.bfloat16
AF = mybir.ActivationFunctionType
OP = mybir.AluOpType

B, S, D = 2, 2048, 1024
H, DK, FF = 16, 64, 2048
EPS = 1e-5
NCORES = 8
SQ = S * B // NCORES          # 512 own query tokens per core
ND = D // 128                 # 8 feature tiles
NT = S // 128                 # 16 sequence tiles
NTQ = SQ // 128               # 4 own-token tiles
NF = FF // 128                # 16 ff tiles
NKC = S // 512                # 4 key chunks of 512

_BF = ml_dtypes.bfloat16


def _build_nc():
    nc = bacc.Bacc("TRN2", target_bir_lowering=False, debug=False,
                   num_devices=NCORES)

    x = nc.dram_tensor("x", [S, D], F32, kind="ExternalInput").ap()
    wqt = nc.dram_tensor("wqt", [D, D], BF16, kind="ExternalInput").ap()
    wkt = nc.dram_tensor("wkt", [D, D], BF16, kind="ExternalInput").ap()
    wvt = nc.dram_tensor("wvt", [D, D], BF16, kind="ExternalInput").ap()
    wot = nc.dram_tensor("wot", [D, D], BF16, kind="ExternalInput").ap()
    w1t = nc.dram_tensor("w1t", [D, FF], BF16, kind="ExternalInput").ap()
    w2t = nc.dram_tensor("w2t", [FF, D], BF16, kind="ExternalInput").ap()
    bqd = nc.dram_tensor("bq", [128, ND], F32, kind="ExternalInput").ap()
    bkd = nc.dram_tensor("bk", [128, ND], F32, kind="ExternalInput").ap()
    bvd = nc.dram_tensor("bv", [1, D], F32, kind="ExternalInput").ap()
    bod = nc.dram_tensor("bo", [128, ND], F32, kind="ExternalInput").ap()
    b1d = nc.dram_tensor("b1", [128, NF], F32, kind="ExternalInput").ap()
    b2d = nc.dram_tensor("b2", [128, ND], F32, kind="ExternalInput").ap()
    outd = nc.dram_tensor("out", [SQ, D], F32, kind="ExternalOutput").ap()

    with tile.TileContext(nc) as tc, ExitStack() as ctx:
        # ---- whole-kernel pools (stack level 0) ----
        const = ctx.enter_context(tc.tile_pool(name="const", bufs=1))
        wA = ctx.enter_context(tc.tile_pool(name="wA", bufs=16))
        xrp = ctx.enter_context(tc.tile_pool(name="xrp", bufs=1))

        ident_f = const.tile([128, 128], F32, tag="idf")
        make_identity(nc, ident_f)
        eps_sb = const.tile([128, 1], F32, tag="eps")
        nc.vector.memset(eps_sb, EPS)
        bq_sb = const.tile([128, ND], F32, tag="bq")
        nc.sync.dma_start(out=bq_sb, in_=bqd)
        bk_sb = const.tile([128, ND], F32, tag="bk")
        nc.sync.dma_start(out=bk_sb, in_=bkd)
        bo_sb = const.tile([128, ND], F32, tag="bo")
        nc.sync.dma_start(out=bo_sb, in_=bod)
        b1_sb = const.tile([128, NF], F32, tag="b1")
        nc.sync.dma_start(out=b1_sb, in_=b1d)
        b2_sb = const.tile([128, ND], F32, tag="b2")
        nc.sync.dma_start(out=b2_sb, in_=b2d)
        bv_row = const.tile([1, D], F32, tag="bvr")
        nc.sync.dma_start(out=bv_row, in_=bvd)
        bv_bc = const.tile([128, D], F32, tag="bvb")
        nc.gpsimd.partition_broadcast(bv_bc, bv_row)

        x_res = [xrp.tile([128, D], F32, tag=f"xr{t}", name=f"xr{t}")
                 for t in range(NTQ)]

        # ---- level 1: K/V/Q/O live from QKV through out-projection ----
        ctxKVQ = ExitStack()
        kp = ctxKVQ.enter_context(tc.tile_pool(name="kp", bufs=1))
        vp = ctxKVQ.enter_context(tc.tile_pool(name="vp", bufs=1))
        qp = ctxKVQ.enter_context(tc.tile_pool(name="qp", bufs=1))
        op_ = ctxKVQ.enter_context(tc.tile_pool(name="op", bufs=1))
        k_fm = [kp.tile([128, S], BF16, tag=f"k{d}", name=f"k{d}")
                for d in range(ND)]
        v_aug = [vp.tile([128, H, DK + 1], BF16, tag=f"v{t}", name=f"v{t}")
                 for t in range(NT)]
        q_fm = [qp.tile([128, SQ], BF16, tag=f"q{d}", name=f"q{d}")
                for d in range(ND)]
        o_fm = [op_.tile([128, SQ], BF16, tag=f"o{j}", name=f"o{j}")
                for j in range(ND)]

        # ---- level 2: LN1 + z (freed right after QKV) ----
        ctxZ = ExitStack()
        zp = ctxZ.enter_context(tc.tile_pool(name="zp", bufs=1))
        xin = ctxZ.enter_context(tc.tile_pool(name="xin", bufs=2))
        lns = ctxZ.enter_context(tc.tile_pool(name="lns", bufs=4))
        psA = ctxZ.enter_context(tc.tile_pool(name="psA", bufs=4,
                                              space="PSUM"))
        # zf[p, j, t]: feature-major z, zf[:, j, :] holds features j*128+p
        zf = zp.tile([128, ND, S], BF16, tag="zf", name="zf")

        for t in range(NT):
            x_t = xin.tile([128, D], F32, tag="x")
            nc.sync.dma_start(out=x_t, in_=x[t * 128:(t + 1) * 128, :])
            st = lns.tile([128, 2, 6], F32, tag="st")
            nc.vector.bn_stats(st[:, 0, :], x_t[:, 0:512])
            nc.vector.bn_stats(st[:, 1, :], x_t[:, 512:1024])
            mv = lns.tile([128, 2], F32, tag="mv")
            nc.vector.bn_aggr(mv, st)
            sq = lns.tile([128, 1], F32, tag="sq")
            nc.scalar.activation(sq, mv[:, 1:2], AF.Sqrt, bias=eps_sb[:, 0:1],
                                 scale=1.0)
            rstd = lns.tile([128, 1], F32, tag="rstd")
            nc.vector.reciprocal(rstd, sq)
            z_tm = lns.tile([128, D], BF16, tag="ztm", bufs=3)
            nc.vector.tensor_scalar(z_tm[:, 0:512], x_t[:, 0:512],
                                    mv[:, 0:1], rstd,
                                    op0=OP.subtract, op1=OP.mult)
            nc.gpsimd.tensor_scalar(z_tm[:, 512:1024], x_t[:, 512:1024],
                                    mv[:, 0:1], rstd,
                                    op0=OP.subtract, op1=OP.mult)
            nc.sync.dma_start_transpose(zf[:, :, t * 128:(t + 1) * 128], z_tm)

        # ---- QKV projections ----
        wq_sb = []
        for d in range(ND):
            w = wA.tile([128, D], BF16, tag="wA", name=f"wq{d}")
            nc.sync.dma_start(out=w, in_=wqt[d * 128:(d + 1) * 128, :])
            wq_sb.append(w)
        wk_sb = []
        for d in range(ND):
            w = wA.tile([128, D], BF16, tag="wA", name=f"wk{d}")
            nc.sync.dma_start(out=w, in_=wkt[d * 128:(d + 1) * 128, :])
            wk_sb.append(w)

        # Q: own 512 tokens
        for j in range(ND):
            pq = psA.tile([128, 512], F32, tag="ps")
            for d in range(ND):
                nc.tensor.matmul(pq, wq_sb[d][:, j * 128:(j + 1) * 128],
                                 zf[:, d, 0:SQ],
                                 start=(d == 0), stop=(d == ND - 1))
            nc.vector.tensor_scalar(q_fm[j], pq, bq_sb[:, j:j + 1], None,
                                    op0=OP.add)

        # K: full 2048 tokens
        for j in range(ND):
            for c in range(NKC):
                pk = psA.tile([128, 512], F32, tag="ps")
                for d in range(ND):
                    nc.tensor.matmul(pk, wk_sb[d][:, j * 128:(j + 1) * 128],
                                     zf[:, d, c * 512:(c + 1) * 512],
                                     start=(d == 0), stop=(d == ND - 1))
                nc.vector.tensor_scalar(k_fm[j][:, c * 512:(c + 1) * 512], pk,
                                        bk_sb[:, j:j + 1], None, op0=OP.add)

        wv_sb = []
        for d in range(ND):
            w = wA.tile([128, D], BF16, tag="wA", name=f"wv{d}")
            nc.sync.dma_start(out=w, in_=wvt[d * 128:(d + 1) * 128, :])
            wv_sb.append(w)

        # V: token-major with appended ones column (softmax denominator)
        for t in range(NT):
            nc.vector.memset(v_aug[t][:, :, DK:DK + 1], 1.0)
            for c in range(2):
                pv = psA.tile([128, 512], F32, tag="ps")
                for d in range(ND):
                    nc.tensor.matmul(pv, zf[:, d, t * 128:(t + 1) * 128],
                                     wv_sb[d][:, c * 512:(c + 1) * 512],
                                     start=(d == 0), stop=(d == ND - 1))
                nc.vector.tensor_add(
                    v_aug[t][:, c * 8:(c + 1) * 8, 0:DK],
                    pv.rearrange("p (h d) -> p h d", h=8),
                    bv_bc[:, c * 512:(c + 1) * 512].rearrange(
                        "p (h d) -> p h d", h=8))
        ctxZ.close()

        # ---- attention (transposed scores, no max subtraction) ----
        ctxATT = ExitStack()
        stp = ctxATT.enter_context(tc.tile_pool(name="stp", bufs=4))
        pgp = ctxATT.enter_context(tc.tile_pool(name="pgp", bufs=3,
                                                space="PSUM"))
        ppvp = ctxATT.enter_context(tc.tile_pool(name="ppvp", bufs=2,
                                                 space="PSUM"))

        wo_sb = []
        for d in range(ND):
            w = wA.tile([128, D], BF16, tag="wA", name=f"wo{d}")
            nc.sync.dma_start(out=w, in_=wot[d * 128:(d + 1) * 128, :])
            wo_sb.append(w)

        for hp in range(H // 2):
            ppv = [ppvp.tile([DK + 1, 512], F32, tag="ppv",
                             name=f"ppv{hp}_{i}") for i in range(2)]
            prev_st = None
            for kt in range(NT + 1):
                if kt < NT:
                    pg = pgp.tile([128, 2, 512], F32, tag="pg")
                    nc.tensor.matmul(
                        pg[:, 0, :],
                        k_fm[hp][0:64, kt * 128:(kt + 1) * 128],
                        q_fm[hp][0:64, :], start=True, stop=True)
                    nc.tensor.matmul(
                        pg[:, 1, :],
                        k_fm[hp][64:128, kt * 128:(kt + 1) * 128],
                        q_fm[hp][64:128, :], start=True, stop=True)
                    stg = stp.tile([128, 2, 512], BF16, tag="st")
                    nc.scalar.activation(stg, pg, AF.Exp, bias=0.0,
                                         scale=0.125)
                if kt > 0:
                    for s in range(2):
                        nc.tensor.matmul(ppv[s],
                                         v_aug[kt - 1][:, 2 * hp + s, :],
                                         prev_st[:, s, :],
                                         start=(kt == 1), stop=(kt == NT))
                prev_st = stg
            for s in range(2):
                nc.vector.tensor_copy(o_fm[hp][s * 64:(s + 1) * 64, :],
                                      ppv[s][0:DK, :])
                den_c = stp.tile([1, 512], F32, tag="denc", bufs=2)
                nc.vector.tensor_copy(den_c, ppv[s][DK:DK + 1, :])
                den_r = stp.tile([1, 512], F32, tag="denr", bufs=2)
                nc.vector.reciprocal(den_r, den_c)
                rb = stp.tile([128, 512], F32, tag="rb", bufs=2)
                nc.gpsimd.partition_broadcast(rb, den_r)
                nc.vector.tensor_mul(o_fm[hp][s * 64:(s + 1) * 64, :],
                                     o_fm[hp][s * 64:(s + 1) * 64, :],
                                     rb[s * 64:(s + 1) * 64, :])
        ctxATT.close()

        # ---- normalize O, output projection + residual (x_res = xq + y) ----
        ctxY = ExitStack()
        xqy = ctxY.enter_context(tc.tile_pool(name="xqy", bufs=1))
        ytp = ctxY.enter_context(tc.tile_pool(name="ytp", bufs=2))
        psB = ctxY.enter_context(tc.tile_pool(name="psB", bufs=2,
                                              space="PSUM"))
        ptY = ctxY.enter_context(tc.tile_pool(name="ptY", bufs=4,
                                              space="PSUM"))

        xq = [xqy.tile([128, D], F32, tag=f"xq{t}", name=f"xq{t}")
              for t in range(NTQ)]
        for t in range(NTQ):
            nc.sync.dma_start(out=xq[t], in_=x[t * 128:(t + 1) * 128, :])

        for o in range(ND):
            py = psB.tile([128, 512], F32, tag="psb")
            for j in range(ND):
                nc.tensor.matmul(py, wo_sb[j][:, o * 128:(o + 1) * 128],
                                 o_fm[j], start=(j == 0), stop=(j == ND - 1))
            y_tmp = ytp.tile([128, 512], F32, tag="yt")
            nc.vector.tensor_scalar(y_tmp, py, bo_sb[:, o:o + 1], None,
                                    op0=OP.add)
            for t in range(NTQ):
                tp = ptY.tile([128, 128], F32, tag="pty")
                nc.tensor.transpose(tp, y_tmp[:, t * 128:(t + 1) * 128],
                                    ident_f)
                nc.vector.tensor_add(x_res[t][:, o * 128:(o + 1) * 128], tp,
                                     xq[t][:, o * 128:(o + 1) * 128])
        ctxY.close()
        ctxKVQ.close()

        # ---- LN2 + MLP + residual ----
        ctxMLP = ExitStack()
        z2p = ctxMLP.enter_context(tc.tile_pool(name="z2p", bufs=1))
        hp_ = ctxMLP.enter_context(tc.tile_pool(name="hp", bufs=1))
        wB = ctxMLP.enter_context(tc.tile_pool(name="wB", bufs=9))
        lns2 = ctxMLP.enter_context(tc.tile_pool(name="lns2", bufs=3))
        y2tp = ctxMLP.enter_context(tc.tile_pool(name="y2tp", bufs=2))
        outp = ctxMLP.enter_context(tc.tile_pool(name="outp", bufs=1))
        psC = ctxMLP.enter_context(tc.tile_pool(name="psC", bufs=4,
                                                space="PSUM"))
        ptM = ctxMLP.enter_context(tc.tile_pool(name="ptM", bufs=4,
                                                space="PSUM"))
        z2f = z2p.tile([128, ND, SQ], BF16, tag="z2f", name="z2f")
        h_fm = [hp_.tile([128, SQ], BF16, tag=f"h{f}", name=f"h{f}")
                for f in range(NF)]
        out_tm = [outp.tile([128, D], F32, tag=f"ot{t}", name=f"ot{t}")
                  for t in range(NTQ)]

        w1_sb = []
        for d in range(ND):
            w = wB.tile([128, FF], BF16, tag="wB", name=f"w1_{d}")
            nc.sync.dma_start(out=w, in_=w1t[d * 128:(d + 1) * 128, :])
            w1_sb.append(w)

        for t in range(NTQ):
            st = lns2.tile([128, 2, 6], F32, tag="st2")
            nc.vector.bn_stats(st[:, 0, :], x_res[t][:, 0:512])
            nc.vector.bn_stats(st[:, 1, :], x_res[t][:, 512:1024])
            mv = lns2.tile([128, 2], F32, tag="mv2")
            nc.vector.bn_aggr(mv, st)
            sq = lns2.tile([128, 1], F32, tag="sq2")
            nc.scalar.activation(sq, mv[:, 1:2], AF.Sqrt, bias=eps_sb[:, 0:1],
                                 scale=1.0)
            rstd = lns2.tile([128, 1], F32, tag="rstd2")
            nc.vector.reciprocal(rstd, sq)
            z2_tm = lns2.tile([128, D], BF16, tag="z2tm")
            nc.vector.tensor_scalar(z2_tm[:, 0:512], x_res[t][:, 0:512],
                                    mv[:, 0:1], rstd,
                                    op0=OP.subtract, op1=OP.mult)
            nc.gpsimd.tensor_scalar(z2_tm[:, 512:1024], x_res[t][:, 512:1024],
                                    mv[:, 0:1], rstd,
                                    op0=OP.subtract, op1=OP.mult)
            nc.sync.dma_start_transpose(z2f[:, :, t * 128:(t + 1) * 128],
                                        z2_tm)

        w2_sb = []
        for f in range(NF):
            w = wA.tile([128, D], BF16, tag="wA", name=f"w2_{f}")
            nc.sync.dma_start(out=w, in_=w2t[f * 128:(f + 1) * 128, :])
            w2_sb.append(w)

        for f in range(NF):
            ph = psC.tile([128, 512], F32, tag="psc")
            for d in range(ND):
                nc.tensor.matmul(ph, w1_sb[d][:, f * 128:(f + 1) * 128],
                                 z2f[:, d, :], start=(d == 0),
                                 stop=(d == ND - 1))
            nc.scalar.activation(h_fm[f], ph, AF.Relu, bias=b1_sb[:, f:f + 1],
                                 scale=1.0)

        for o in range(ND):
            p2 = psC.tile([128, 512], F32, tag="psc")
            for f in range(NF):
                nc.tensor.matmul(p2, w2_sb[f][:, o * 128:(o + 1) * 128],
                                 h_fm[f], start=(f == 0), stop=(f == NF - 1))
            y2_tmp = y2tp.tile([128, 512], F32, tag="y2t")
            nc.vector.tensor_scalar(y2_tmp, p2, b2_sb[:, o:o + 1], None,
                                    op0=OP.add)
            for t in range(NTQ):
                tp = ptM.tile([128, 128], F32, tag="ptm")
                nc.tensor.transpose(tp, y2_tmp[:, t * 128:(t + 1) * 128],
                                    ident_f)
                nc.vector.tensor_add(out_tm[t][:, o * 128:(o + 1) * 128], tp,
                                     x_res[t][:, o * 128:(o + 1) * 128])
        for t in range(NTQ):
            nc.sync.dma_start(out=outd[t * 128:(t + 1) * 128, :],
                              in_=out_tm[t])
        ctxMLP.close()

    nc.compile()
    return nc


_LOCK = threading.Lock()
_NC = None


def _get_nc():
    global _NC
    with _LOCK:
        if _NC is None:
            _NC = _build_nc()
    return _NC


def _prep_inputs(inputs):
    x = np.asarray(inputs["x"], np.float32)
    g1 = np.asarray(inputs["ln1_g"], np.float32)
    b1v = np.asarray(inputs["ln1_b"], np.float32)
    g2 = np.asarray(inputs["ln2_g"], np.float32)
    b2v = np.asarray(inputs["ln2_b"], np.float32)
    wq = np.asarray(inputs["wq"], np.float32)
    wk = np.asarray(inputs["wk"], np.float32)
    wv = np.asarray(inputs["wv"], np.float32)
    wo = np.asarray(inputs["wo"], np.float32)
    w1 = np.asarray(inputs["w1"], np.float32)
    w2 = np.asarray(inputs["w2"], np.float32)

    shared = {
        "wqt": np.ascontiguousarray((g1[:, None] * wq.T)).astype(_BF),
        "wkt": np.ascontiguousarray((g1[:, None] * wk.T)).astype(_BF),
        "wvt": np.ascontiguousarray((g1[:, None] * wv.T)).astype(_BF),
        "wot": np.ascontiguousarray(wo.T).astype(_BF),
        "w1t": np.ascontiguousarray((g2[:, None] * w1.T)).astype(_BF),
        "w2t": np.ascontiguousarray(w2.T).astype(_BF),
        "bq": np.ascontiguousarray(
            (inputs["bq"] + wq @ b1v).astype(np.float32).reshape(ND, 128).T),
        "bk": np.ascontiguousarray(
            (inputs["bk"] + wk @ b1v).astype(np.float32).reshape(ND, 128).T),
        "bv": (inputs["bv"] + wv @ b1v).astype(np.float32).reshape(1, D),
        "bo": np.ascontiguousarray(
            np.asarray(inputs["bo"], np.float32).reshape(ND, 128).T),
        "b1": np.ascontiguousarray(
            (inputs["b1"] + w1 @ b2v).astype(np.float32).reshape(NF, 128).T),
        "b2": np.ascontiguousarray(
            np.asarray(inputs["b2"], np.float32).reshape(ND, 128).T),
    }

    in_maps = []
    for c in range(NCORES):
        b = c // (NCORES // B)
        qoff = (c % (NCORES // B)) * SQ
        xb = x[b]
        x_perm = np.ascontiguousarray(
            np.concatenate([xb[qoff:qoff + SQ], xb[:qoff], xb[qoff + SQ:]],
                           axis=0))
        m = dict(shared)
        m["x"] = x_perm
        in_maps.append(m)
    return in_maps


def _run(inputs, trace=False):
    nc = _get_nc()
    in_maps = _prep_inputs(inputs)
    res = run_bass_kernel_spmd(nc, in_maps, core_ids=list(range(NCORES)),
                               trace=trace)
    out = np.empty((B, S, D), np.float32)
    for c in range(NCORES):
        b = c // (NCORES // B)
        qoff = (c % (NCORES // B)) * SQ
        out[b, qoff:qoff + SQ] = res.results[c]["out"]
    return out, res


def kernel(**inputs):
    out, _ = _run(inputs, trace=False)
    return out


# revision 8
# speedup vs baseline: 1.2513x; 1.1635x over previous
"""Trainium2 Bass kernel for a pre-norm transformer decoder layer.

Full inputs in, full output out. Internally: 8-way data-parallel over
tokens (batch 2 x 4 query-slices of 512 tokens). Each core redundantly
computes K/V for its batch's full 2048-token sequence (no collectives),
and owns 512 query tokens end-to-end (attention, out-proj, MLP).

Shapes: x (2, 2048, 1024), 16 heads, dk=64, d_ff=2048, eps=1e-5.
"""
import threading

import numpy as np
import ml_dtypes

import concourse.mybir as mybir
import concourse.tile as tile
from concourse import bacc
from concourse.bass_utils import run_bass_kernel_spmd
from concourse.masks import make_identity
from contextlib import ExitStack

F32 = mybir.dt.float32
BF16 = mybir.dt.bfloat16
AF = mybir.ActivationFunctionType
OP = mybir.AluOpType

B, S, D = 2, 2048, 1024
H, DK, FF = 16, 64, 2048
EPS = 1e-5
NCORES = 8
SQ = S * B // NCORES          # 512 own query tokens per core
ND = D // 128                 # 8 feature tiles
NT = S // 128                 # 16 sequence tiles
NTQ = SQ // 128               # 4 own-token tiles
NF = FF // 128                # 16 ff tiles
NKC = S // 512                # 4 key chunks of 512

_BF = ml_dtypes.bfloat16


def _build_nc():
    nc = bacc.Bacc("TRN2", target_bir_lowering=False, debug=False,
                   num_devices=NCORES)

    x = nc.dram_tensor("x", [S, D], F32, kind="ExternalInput").ap()
    wqt = nc.dram_tensor("wqt", [D, D], BF16, kind="ExternalInput").ap()
    wkt = nc.dram_tensor("wkt", [D, D], BF16, kind="ExternalInput").ap()
    wvt = nc.dram_tensor("wvt", [D, D], BF16, kind="ExternalInput").ap()
    wot = nc.dram_tensor("wot", [D, D], BF16, kind="ExternalInput").ap()
    w1t = nc.dram_tensor("w1t", [D, FF], BF16, kind="ExternalInput").ap()
    w2t = nc.dram_tensor("w2t", [FF, D], BF16, kind="ExternalInput").ap()
    bqd = nc.dram_tensor("bq", [128, ND], F32, kind="ExternalInput").ap()
    bkd = nc.dram_tensor("bk", [128, ND], F32, kind="ExternalInput").ap()
    bvd = nc.dram_tensor("bv", [1, D], F32, kind="ExternalInput").ap()
    bod = nc.dram_tensor("bo", [128, ND], F32, kind="ExternalInput").ap()
    b1d = nc.dram_tensor("b1", [128, NF], F32, kind="ExternalInput").ap()
    b2d = nc.dram_tensor("b2", [128, ND], F32, kind="ExternalInput").ap()
    outd = nc.dram_tensor("out", [SQ, D], F32, kind="ExternalOutput").ap()

    with tile.TileContext(nc) as tc, ExitStack() as ctx:
        # ---- whole-kernel pools (stack level 0) ----
        const = ctx.enter_context(tc.tile_pool(name="const", bufs=1))
        wA = ctx.enter_context(tc.tile_pool(name="wA", bufs=16))
        xrp = ctx.enter_context(tc.tile_pool(name="xrp", bufs=1))

        ident_f = const.tile([128, 128], F32, tag="idf")
        make_identity(nc, ident_f)
        eps_sb = const.tile([128, 1], F32, tag="eps")
        nc.vector.memset(eps_sb, EPS)
        bq_sb = const.tile([128, ND], F32, tag="bq")
        nc.sync.dma_start(out=bq_sb, in_=bqd)
        bk_sb = const.tile([128, ND], F32, tag="bk")
        nc.sync.dma_start(out=bk_sb, in_=bkd)
        bo_sb = const.tile([128, ND], F32, tag="bo")
        nc.sync.dma_start(out=bo_sb, in_=bod)
        b1_sb = const.tile([128, NF], F32, tag="b1")
        nc.sync.dma_start(out=b1_sb, in_=b1d)
        b2_sb = const.tile([128, ND], F32, tag="b2")
        nc.sync.dma_start(out=b2_sb, in_=b2d)
        bv_row = const.tile([1, D], F32, tag="bvr")
        nc.sync.dma_start(out=bv_row, in_=bvd)
        bv_bc = const.tile([128, D], F32, tag="bvb")
        nc.gpsimd.partition_broadcast(bv_bc, bv_row)

        x_res = [xrp.tile([128, D], F32, tag=f"xr{t}", name=f"xr{t}")
                 for t in range(NTQ)]

        # ---- level 1: K/V/Q/O live from QKV through out-projection ----
        ctxKVQ = ExitStack()
        kp = ctxKVQ.enter_context(tc.tile_pool(name="kp", bufs=1))
        vp = ctxKVQ.enter_context(tc.tile_pool(name="vp", bufs=1))
        qp = ctxKVQ.enter_context(tc.tile_pool(name="qp", bufs=1))
        op_ = ctxKVQ.enter_context(tc.tile_pool(name="op", bufs=1))
        k_fm = [kp.tile([128, S], BF16, tag=f"k{d}", name=f"k{d}")
                for d in range(ND)]
        v_aug = [vp.tile([128, H, DK + 1], BF16, tag=f"v{t}", name=f"v{t}")
                 for t in range(NT)]
        q_fm = [qp.tile([128, SQ], BF16, tag=f"q{d}", name=f"q{d}")
                for d in range(ND)]
        o_fm = [op_.tile([128, SQ], BF16, tag=f"o{j}", name=f"o{j}")
                for j in range(ND)]

        # ---- level 2: LN1 + z (freed right after QKV) ----
        ctxZ = ExitStack()
        zp = ctxZ.enter_context(tc.tile_pool(name="zp", bufs=1))
        xin = ctxZ.enter_context(tc.tile_pool(name="xin", bufs=2))
        lns = ctxZ.enter_context(tc.tile_pool(name="lns", bufs=4))
        psA = ctxZ.enter_context(tc.tile_pool(name="psA", bufs=4,
                                              space="PSUM"))
        # zq[i][p, j, t]: feature-major z quadrant i (tokens i*512..+512);
        # zq[i][:, j, :] holds features j*128+p
        zq = [zp.tile([128, ND, 512], BF16, tag=f"zq{i}", name=f"zq{i}")
              for i in range(4)]

        for t in range(NT):
            x_t = xin.tile([128, D], F32, tag="x")
            nc.sync.dma_start(out=x_t, in_=x[t * 128:(t + 1) * 128, :])
            st = lns.tile([128, 2, 6], F32, tag="st")
            nc.vector.bn_stats(st[:, 0, :], x_t[:, 0:512])
            nc.vector.bn_stats(st[:, 1, :], x_t[:, 512:1024])
            mv = lns.tile([128, 2], F32, tag="mv")
            nc.vector.bn_aggr(mv, st)
            sq = lns.tile([128, 1], F32, tag="sq")
            nc.scalar.activation(sq, mv[:, 1:2], AF.Sqrt, bias=eps_sb[:, 0:1],
                                 scale=1.0)
            rstd = lns.tile([128, 1], F32, tag="rstd")
            nc.vector.reciprocal(rstd, sq)
            z_tm = lns.tile([128, D], BF16, tag="ztm", bufs=3)
            nc.vector.tensor_scalar(z_tm, x_t, mv[:, 0:1], rstd,
                                    op0=OP.subtract, op1=OP.mult)
            nc.sync.dma_start_transpose(
                zq[t // 4][:, :, (t % 4) * 128:(t % 4 + 1) * 128], z_tm)

        # ---- QKV projections ----
        wq_sb = []
        for d in range(ND):
            w = wA.tile([128, D], BF16, tag="wA", name=f"wq{d}")
            nc.scalar.dma_start(out=w, in_=wqt[d * 128:(d + 1) * 128, :])
            wq_sb.append(w)
        wk_sb = []
        for d in range(ND):
            w = wA.tile([128, D], BF16, tag="wA", name=f"wk{d}")
            nc.scalar.dma_start(out=w, in_=wkt[d * 128:(d + 1) * 128, :])
            wk_sb.append(w)

        # Q: own 512 tokens
        for j in range(ND):
            pq = psA.tile([128, 512], F32, tag="ps")
            for d in range(ND):
                nc.tensor.matmul(pq, wq_sb[d][:, j * 128:(j + 1) * 128],
                                 zq[0][:, d, :],
                                 start=(d == 0), stop=(d == ND - 1))
            nc.vector.tensor_scalar(q_fm[j], pq, bq_sb[:, j:j + 1], None,
                                    op0=OP.add)

        wv_sb = []
        for d in range(ND):
            w = wA.tile([128, D], BF16, tag="wA", name=f"wv{d}")
            nc.scalar.dma_start(out=w, in_=wvt[d * 128:(d + 1) * 128, :])
            wv_sb.append(w)

        # K (full 2048 tokens) and V (token-major with appended ones column
        # for the softmax denominator), interleaved by z quadrant
        for c in range(NKC):
            for j in range(ND):
                pk = psA.tile([128, 512], F32, tag="ps")
                for d in range(ND):
                    nc.tensor.matmul(pk, wk_sb[d][:, j * 128:(j + 1) * 128],
                                     zq[c][:, d, :],
                                     start=(d == 0), stop=(d == ND - 1))
                nc.vector.tensor_scalar(k_fm[j][:, c * 512:(c + 1) * 512], pk,
                                        bk_sb[:, j:j + 1], None, op0=OP.add)
            for t in range(4 * c, 4 * c + 4):
                nc.vector.memset(v_aug[t][:, :, DK:DK + 1], 1.0)
                for ch in range(2):
                    pv = psA.tile([128, 512], F32, tag="ps")
                    for d in range(ND):
                        nc.tensor.matmul(
                            pv, zq[c][:, d, (t % 4) * 128:(t % 4 + 1) * 128],
                            wv_sb[d][:, ch * 512:(ch + 1) * 512],
                            start=(d == 0), stop=(d == ND - 1))
                    nc.vector.tensor_add(
                        v_aug[t][:, ch * 8:(ch + 1) * 8, 0:DK],
                        pv.rearrange("p (h d) -> p h d", h=8),
                        bv_bc[:, ch * 512:(ch + 1) * 512].rearrange(
                            "p (h d) -> p h d", h=8))
        ctxZ.close()

        # ---- attention (transposed scores, no max subtraction) ----
        ctxATT = ExitStack()
        stp = ctxATT.enter_context(tc.tile_pool(name="stp", bufs=4))
        pgp = ctxATT.enter_context(tc.tile_pool(name="pgp", bufs=3,
                                                space="PSUM"))
        ppvp = ctxATT.enter_context(tc.tile_pool(name="ppvp", bufs=2,
                                                 space="PSUM"))

        wo_sb = []
        for d in range(ND):
            w = wA.tile([128, D], BF16, tag="wA", name=f"wo{d}")
            nc.scalar.dma_start(out=w, in_=wot[d * 128:(d + 1) * 128, :])
            wo_sb.append(w)

        for hp in range(H // 2):
            ppv = [ppvp.tile([DK + 1, 512], F32, tag="ppv",
                             name=f"ppv{hp}_{i}") for i in range(2)]
            prev_st = None
            for kt in range(NT + 1):
                if kt < NT:
                    pg = pgp.tile([128, 2, 512], F32, tag="pg")
                    nc.tensor.matmul(
                        pg[:, 0, :],
                        k_fm[hp][0:64, kt * 128:(kt + 1) * 128],
                        q_fm[hp][0:64, :], start=True, stop=True)
                    nc.tensor.matmul(
                        pg[:, 1, :],
                        k_fm[hp][64:128, kt * 128:(kt + 1) * 128],
                        q_fm[hp][64:128, :], start=True, stop=True)
                    stg = stp.tile([128, 2, 512], BF16, tag="st")
                    nc.scalar.activation(stg, pg, AF.Exp, bias=0.0,
                                         scale=0.125)
                if kt > 0:
                    for s in range(2):
                        nc.tensor.matmul(ppv[s],
                                         v_aug[kt - 1][:, 2 * hp + s, :],
                                         prev_st[:, s, :],
                                         start=(kt == 1), stop=(kt == NT))
                prev_st = stg
            for s in range(2):
                nc.vector.tensor_copy(o_fm[hp][s * 64:(s + 1) * 64, :],
                                      ppv[s][0:DK, :])
                den_c = stp.tile([1, 512], F32, tag="denc", bufs=2)
                nc.vector.tensor_copy(den_c, ppv[s][DK:DK + 1, :])
                den_r = stp.tile([1, 512], F32, tag="denr", bufs=2)
                nc.vector.reciprocal(den_r, den_c)
                rb = stp.tile([128, 512], F32, tag="rb", bufs=2)
                nc.gpsimd.partition_broadcast(rb, den_r)
                nc.vector.tensor_mul(o_fm[hp][s * 64:(s + 1) * 64, :],
                                     o_fm[hp][s * 64:(s + 1) * 64, :],
                                     rb[s * 64:(s + 1) * 64, :])
        ctxATT.close()

        # ---- normalize O, output projection + residual (x_res = xq + y) ----
        ctxY = ExitStack()
        xqy = ctxY.enter_context(tc.tile_pool(name="xqy", bufs=1))
        ytp = ctxY.enter_context(tc.tile_pool(name="ytp", bufs=2))
        psB = ctxY.enter_context(tc.tile_pool(name="psB", bufs=3,
                                              space="PSUM"))

        xq = [xqy.tile([128, D], F32, tag=f"xq{t}", name=f"xq{t}")
              for t in range(NTQ)]
        for t in range(NTQ):
            nc.sync.dma_start(out=xq[t], in_=x[t * 128:(t + 1) * 128, :])
        # y_tm[p, t, o]: token-major attention output (bf16 via xbar)
        y_tm = xqy.tile([128, NTQ, D], BF16, tag="ytm", name="y_tm")

        for o in range(ND):
            py = psB.tile([128, 512], F32, tag="psb")
            for j in range(ND):
                nc.tensor.matmul(py, wo_sb[j][:, o * 128:(o + 1) * 128],
                                 o_fm[j], start=(j == 0), stop=(j == ND - 1))
            y_tmp = ytp.tile([128, 512], BF16, tag="yt")
            nc.vector.tensor_scalar(y_tmp, py, bo_sb[:, o:o + 1], None,
                                    op0=OP.add)
            nc.sync.dma_start_transpose(y_tm[:, :, o * 128:(o + 1) * 128],
                                        y_tmp)
        for t in range(NTQ):
            nc.vector.tensor_add(x_res[t], y_tm[:, t, :], xq[t])
        ctxY.close()
        ctxKVQ.close()

        # ---- LN2 + MLP + residual ----
        ctxMLP = ExitStack()
        z2p = ctxMLP.enter_context(tc.tile_pool(name="z2p", bufs=1))
        hp_ = ctxMLP.enter_context(tc.tile_pool(name="hp", bufs=1))
        wB = ctxMLP.enter_context(tc.tile_pool(name="wB", bufs=9))
        lns2 = ctxMLP.enter_context(tc.tile_pool(name="lns2", bufs=3))
        y2tp = ctxMLP.enter_context(tc.tile_pool(name="y2tp", bufs=2))
        outp = ctxMLP.enter_context(tc.tile_pool(name="outp", bufs=1))
        psC = ctxMLP.enter_context(tc.tile_pool(name="psC", bufs=4,
                                                space="PSUM"))
        z2f = z2p.tile([128, ND, SQ], BF16, tag="z2f", name="z2f")
        h_fm = [hp_.tile([128, SQ], BF16, tag=f"h{f}", name=f"h{f}")
                for f in range(NF)]
        out_tm = [outp.tile([128, D], F32, tag=f"ot{t}", name=f"ot{t}")
                  for t in range(NTQ)]

        w1_sb = []
        for d in range(ND):
            w = wB.tile([128, FF], BF16, tag="wB", name=f"w1_{d}")
            nc.scalar.dma_start(out=w, in_=w1t[d * 128:(d + 1) * 128, :])
            w1_sb.append(w)

        for t in range(NTQ):
            st = lns2.tile([128, 2, 6], F32, tag="st2")
            nc.vector.bn_stats(st[:, 0, :], x_res[t][:, 0:512])
            nc.vector.bn_stats(st[:, 1, :], x_res[t][:, 512:1024])
            mv = lns2.tile([128, 2], F32, tag="mv2")
            nc.vector.bn_aggr(mv, st)
            sq = lns2.tile([128, 1], F32, tag="sq2")
            nc.scalar.activation(sq, mv[:, 1:2], AF.Sqrt, bias=eps_sb[:, 0:1],
                                 scale=1.0)
            rstd = lns2.tile([128, 1], F32, tag="rstd2")
            nc.vector.reciprocal(rstd, sq)
            z2_tm = lns2.tile([128, D], BF16, tag="z2tm")
            nc.vector.tensor_scalar(z2_tm, x_res[t], mv[:, 0:1], rstd,
                                    op0=OP.subtract, op1=OP.mult)
            nc.sync.dma_start_transpose(z2f[:, :, t * 128:(t + 1) * 128],
                                        z2_tm)

        w2_sb = []
        for f in range(NF):
            w = wA.tile([128, D], BF16, tag="wA", name=f"w2_{f}")
            nc.scalar.dma_start(out=w, in_=w2t[f * 128:(f + 1) * 128, :])
            w2_sb.append(w)

        for f in range(NF):
            ph = psC.tile([128, 512], F32, tag="psc")
            for d in range(ND):
                nc.tensor.matmul(ph, w1_sb[d][:, f * 128:(f + 1) * 128],
                                 z2f[:, d, :], start=(d == 0),
                                 stop=(d == ND - 1))
            nc.scalar.activation(h_fm[f], ph, AF.Relu, bias=b1_sb[:, f:f + 1],
                                 scale=1.0)

        y2_tm = outp.tile([128, NTQ, D], BF16, tag="y2tm", name="y2_tm")
        for o in range(ND):
            p2 = psC.tile([128, 512], F32, tag="psc")
            for f in range(NF):
                nc.tensor.matmul(p2, w2_sb[f][:, o * 128:(o + 1) * 128],
                                 h_fm[f], start=(f == 0), stop=(f == NF - 1))
            y2_tmp = y2tp.tile([128, 512], BF16, tag="y2t")
            nc.vector.tensor_scalar(y2_tmp, p2, b2_sb[:, o:o + 1], None,
                                    op0=OP.add)
            nc.sync.dma_start_transpose(y2_tm[:, :, o * 128:(o + 1) * 128],
                                        y2_tmp)
        for t in range(NTQ):
            nc.vector.tensor_add(out_tm[t], y2_tm[:, t, :], x_res[t])
            nc.sync.dma_start(out=outd[t * 128:(t + 1) * 128, :],
                              in_=out_tm[t])
        ctxMLP.close()

    nc.compile()
    return nc


_LOCK = threading.Lock()
_NC = None


def _get_nc():
    global _NC
    with _LOCK:
        if _NC is None:
            _NC = _build_nc()
    return _NC


def _prep_inputs(inputs):
    x = np.asarray(inputs["x"], np.float32)
    g1 = np.asarray(inputs["ln1_g"], np.float32)
    b1v = np.asarray(inputs["ln1_b"], np.float32)
    g2 = np.asarray(inputs["ln2_g"], np.float32)
    b2v = np.asarray(inputs["ln2_b"], np.float32)
    wq = np.asarray(inputs["wq"], np.float32)
    wk = np.asarray(inputs["wk"], np.float32)
    wv = np.asarray(inputs["wv"], np.float32)
    wo = np.asarray(inputs["wo"], np.float32)
    w1 = np.asarray(inputs["w1"], np.float32)
    w2 = np.asarray(inputs["w2"], np.float32)

    shared = {
        "wqt": np.ascontiguousarray((g1[:, None] * wq.T)).astype(_BF),
        "wkt": np.ascontiguousarray((g1[:, None] * wk.T)).astype(_BF),
        "wvt": np.ascontiguousarray((g1[:, None] * wv.T)).astype(_BF),
        "wot": np.ascontiguousarray(wo.T).astype(_BF),
        "w1t": np.ascontiguousarray((g2[:, None] * w1.T)).astype(_BF),
        "w2t": np.ascontiguousarray(w2.T).astype(_BF),
        "bq": np.ascontiguousarray(
            (inputs["bq"] + wq @ b1v).astype(np.float32).reshape(ND, 128).T),
        "bk": np.ascontiguousarray(
            (inputs["bk"] + wk @ b1v).astype(np.float32).reshape(ND, 128).T),
        "bv": (inputs["bv"] + wv @ b1v).astype(np.float32).reshape(1, D),
        "bo": np.ascontiguousarray(
            np.asarray(inputs["bo"], np.float32).reshape(ND, 128).T),
        "b1": np.ascontiguousarray(
            (inputs["b1"] + w1 @ b2v).astype(np.float32).reshape(NF, 128).T),
        "b2": np.ascontiguousarray(
            np.asarray(inputs["b2"], np.float32).reshape(ND, 128).T),
    }

    in_maps = []
    for c in range(NCORES):
        b = c // (NCORES // B)
        qoff = (c % (NCORES // B)) * SQ
        xb = x[b]
        x_perm = np.ascontiguousarray(
            np.concatenate([xb[qoff:qoff + SQ], xb[:qoff], xb[qoff + SQ:]],
                           axis=0))
        m = dict(shared)
        m["x"] = x_perm
        in_maps.append(m)
    return in_maps


def _run(inputs, trace=False):
    nc = _get_nc()
    in_maps = _prep_inputs(inputs)
    res = run_bass_kernel_spmd(nc, in_maps, core_ids=list(range(NCORES)),
                               trace=trace)
    out = np.empty((B, S, D), np.float32)
    for c in range(NCORES):
        b = c // (NCORES // B)
        qoff = (c % (NCORES // B)) * SQ
        out[b, qoff:qoff + SQ] = res.results[c]["out"]
    return out, res


def kernel(**inputs):
    out, _ = _run(inputs, trace=False)
    return out
